# revision 1
# baseline (speedup 1.0000x reference)
"""ConvTranspose2d (16,256,32,32) -> (16,128,66,66), stride 2, 4x4 kernel.

Strategy: data-parallel over batch, 2 images per core on 8 NeuronCores.

Math: y[b,co,2m+p,2n+q] = bias[co]
        + sum_{i,j in {0,1}} sum_ci x[b,ci,m-i,n-j] * w[ci,co,p+2i,q+2j]
for parity class (p,q) in {0,1}^2, m,n in [0,33).

Per image and parity class: output subgrid [128co x 33 x 33] computed as
3 row-chunks of 11 rows; each chunk is one PSUM accumulation group of
8 matmuls (2 ci-chunks x 4 taps (i,j)), K=128, M=128, N=11*33=363,
in float32r (full-rate fp32 on the PE at N>=256).  Shifted taps read a
zero-padded 34x34 SBUF copy of x (padded host-side) through a strided
2D access pattern, so no junk columns are computed.  PSUM->SBUF drain
is a DVE tensor_scalar_add fusing the bias add and the parity
de-interleave.

Overlap choices: weights are DMA'd per parity class in consumption
order; image 0 runs class-major and its output leaves as one DMA that
overlaps image 1's compute; image 1 runs row-band-major and its output
leaves as three 22-row band DMAs so only the last ~0.7MB trails the
final matmul.
"""

import numpy as np

import concourse.bass as bass
import concourse.bacc as bacc
import concourse.tile as tile
from concourse import mybir
from concourse.bass_utils import run_bass_kernel_spmd

N_CORES = 8
B_PER = 2  # images per core

F32 = mybir.dt.float32
F32R = mybir.dt.float32r

PW = 34            # padded x width (32 + 1 left + 1 right)
XLEN = PW * PW     # 1156 padded x elems per partition
XPAD = 1160        # sbuf/dram x free size (AP slack for the last chunk)
R = 11             # output parity rows per PSUM chunk
NCH = 3            # chunks: 3 * 11 = 33 parity rows
NF = R * PW        # 374 matmul free dim (fp32r needs a contiguous rhs,
                   # so the pad column rides along and is dropped on drain)


def _emit_group(nc, ps, wt, xp, p, q, r):
    """One PSUM accumulation group: 8 matmuls for class (p,q), chunk r."""
    m0 = R * r
    k = 0
    for c in range(2):
        for i in range(2):
            for j in range(2):
                off = (m0 - i + 1) * PW + (1 - j)
                nc.tensor.matmul(
                    ps[:],
                    wt[c][:, p, q, i, j, :],
                    xp[c][:, off:off + NF],
                    start=(k == 0),
                    stop=(k == 7),
                )
                k += 1


def _emit_class(nc, pss, wt, xp, p, q, rs):
    """Chunks `rs` of class (p,q), tap-major: consecutive matmuls share
    the stationary weights, so their LDWEIGHTS overlap in-flight matmuls
    instead of gating them.  pss[r] is the PSUM tile for chunk r."""
    k = 0
    for c in range(2):
        for i in range(2):
            for j in range(2):
                for r in rs:
                    off = (R * r - i + 1) * PW + (1 - j)
                    nc.tensor.matmul(
                        pss[r][:],
                        wt[c][:, p, q, i, j, :],
                        xp[c][:, off:off + NF],
                        start=(k == 0),
                        stop=(k == 7),
                        skip_group_check=True,
                    )
                k += 1


def build_nc(debug: bool = False) -> bass.Bass:
    nc = bacc.Bacc("TRN2", target_bir_lowering=False, debug=debug,
                   num_devices=N_CORES)

    # x arrives host-padded: 34x34 zero-border layout + tail pad, flat
    x_d = nc.declare_dram_parameter("x", [B_PER, 256, XPAD], F32R,
                                    isOutput=False)
    # w layout: [ci_chunk, ci, p, q, i, j, co]  (class-major taps)
    w_d = nc.declare_dram_parameter("w", [2, 128, 2, 2, 2, 2, 128], F32R,
                                    isOutput=False)
    b_d = nc.declare_dram_parameter("b", [128, 1], F32, isOutput=False)
    y_d = nc.declare_dram_parameter("y", [B_PER, 128, 66, 66], F32,
                                    isOutput=True)

    with tile.TileContext(nc) as tc:
        with (
            tc.tile_pool(name="wp", bufs=2) as wpool,
            tc.tile_pool(name="bp", bufs=1) as bpool,
            tc.tile_pool(name="xp", bufs=2 * B_PER) as xpool,
            tc.tile_pool(name="yp", bufs=1) as ypool,
            tc.tile_pool(name="ybp", bufs=NCH) as bandpool,
            tc.tile_pool(name="ps", bufs=7, space="PSUM") as ppool,
            tc.tile_pool(name="pw", bufs=1, space="PSUM") as warmpool,
        ):
            # PE warm-up: HAM starts the PE at 1.2GHz and only unthrottles
            # after ~3.4us of sustained activity.  Burn that window on dummy
            # bf16 matmuls during the input-DMA ramp so the real matmuls
            # start at 2.4GHz.
            wub = bpool.tile([128, 512], mybir.dt.bfloat16)
            nc.vector.memset(wub[:], 0.0)
            wps = warmpool.tile([128, 512], F32)
            for _ in range(8):
                nc.tensor.matmul(wps[:], wub[:, 0:128], wub[:],
                                 start=True, stop=True)
            # weight tiles; DMA'd per class in consumption order
            wt = [wpool.tile([128, 2, 2, 2, 2, 128], F32R, name=f"wt{c}", tag="wt")
                  for c in range(2)]

            # ~620ns of sequencer time per dma_start: round-robin the
            # input-DMA issues over four engines so they don't serialize
            issue_engines = [nc.sync, nc.scalar, nc.gpsimd]
            issue_i = [0]

            def dma_in(out, in_):
                eng = issue_engines[issue_i[0] % len(issue_engines)]
                issue_i[0] += 1
                eng.dma_start(out=out, in_=in_)

            def dma_w_class(p, q, per_tap: bool = False):
                for c in range(2):
                    if per_tap:
                        # first-consumed class: land the first 64KB tap
                        # fast so the first real matmul isn't gated on
                        # the whole 512KB class
                        for i in range(2):
                            for j in range(2):
                                dma_in(wt[c][:, p, q, i, j],
                                       w_d[c, :, p, q, i, j])
                    else:
                        dma_in(wt[c][:, p, q], w_d[c, :, p, q])

            xp = {}

            def dma_x(img, banded: bool):
                xp[img] = [
                    xpool.tile([128, XPAD], F32R, name=f"x{img}c{c}",
                               tag="xt")
                    for c in range(2)
                ]
                # row bands [0:13), [13:24), [24:34): chunk r only needs
                # padded rows up to r*R + 12, so compute can start after
                # the first band lands
                bands = [(0, 13 * PW), (13 * PW, 24 * PW), (24 * PW, XPAD)]
                for lo, hi in (bands if banded else [(0, XPAD)]):
                    for c in range(2):
                        dma_in(xp[img][c][:, lo:hi],
                               x_d[img, c * 128:(c + 1) * 128, lo:hi])

            # issue order = consumption order
            dma_w_class(0, 0)
            dma_x(0, banded=True)
            dma_w_class(0, 1)
            dma_w_class(1, 0)
            dma_w_class(1, 1)
            dma_x(1, banded=False)
            bt = bpool.tile([128, 1], F32)
            nc.sync.dma_start(out=bt[:], in_=b_d[:])

            def drain(ps, out_view):
                nc.vector.tensor_scalar_add(
                    out_view,
                    ps[:].rearrange("p (m n) -> p m n", n=PW)[:, :, 0:33],
                    bt[:],
                )

            # ---- image 0: class-major; single output DMA ----
            yt = ypool.tile([128, 66, 66], F32)
            for p in range(2):
                for q in range(2):
                    for r in range(NCH):
                        ps = ppool.tile([128, NF], F32)
                        _emit_group(nc, ps, wt, xp[0], p, q, r)
                        drain(ps, yt[:, p::2, q::2][:, R * r:R * (r + 1), :])
            nc.gpsimd.dma_start(out=y_d[0], in_=yt[:])

            # ---- image 1: band-major; banded output DMAs ----
            for r in range(NCH):
                band = bandpool.tile([128, 2 * R, 66], F32)
                for p in range(2):
                    for q in range(2):
                        ps = ppool.tile([128, NF], F32)
                        _emit_group(nc, ps, wt, xp[1], p, q, r)
                        drain(ps, band[:, p::2, q::2])
                nc.gpsimd.dma_start(
                    out=y_d[1][:, 2 * R * r:2 * R * (r + 1), :],
                    in_=band[:],
                )

    nc.compile()
    return nc


_nc_cache = None


def _get_nc():
    global _nc_cache
    if _nc_cache is None:
        _nc_cache = build_nc()
    return _nc_cache


def make_in_maps(x: np.ndarray, weight: np.ndarray, bias: np.ndarray):
    # w[ci,co,kh,kw] -> [c, ci', p, q, i, j, co]
    w6 = (
        weight.astype(np.float32, copy=False)
        .reshape(2, 128, 128, 2, 2, 2, 2)      # [c, ci', co, i, p, j, q]
        .transpose(0, 1, 4, 6, 3, 5, 2)        # -> [c, ci', p, q, i, j, co]
    )
    w_host = np.ascontiguousarray(w6)
    b_host = np.ascontiguousarray(
        bias.astype(np.float32, copy=False).reshape(128, 1)
    )
    x = np.asarray(x, dtype=np.float32)
    # host-side zero-pad into the 34x34(+tail) layout the kernel reads
    xpad = np.zeros((16, 256, XPAD), dtype=np.float32)
    xpad[:, :, :XLEN].reshape(16, 256, PW, PW)[:, :, 1:33, 1:33] = x
    return [
        {
            "x": np.ascontiguousarray(xpad[B_PER * i:B_PER * (i + 1)]),
            "w": w_host,
            "b": b_host,
        }
        for i in range(N_CORES)
    ]


def kernel(x: np.ndarray, weight: np.ndarray, bias: np.ndarray) -> np.ndarray:
    nc = _get_nc()
    in_maps = make_in_maps(x, weight, bias)
    res = run_bass_kernel_spmd(nc, in_maps, list(range(N_CORES)))
    out = np.concatenate([r["y"] for r in res.results], axis=0)
    return np.ascontiguousarray(out.astype(np.float32, copy=False))



# revision 8
# speedup vs baseline: 1.0916x; 1.0916x over previous
"""ConvTranspose2d (16,256,32,32) -> (16,128,66,66), stride 2, 4x4 kernel.

Strategy: data-parallel over batch, 2 images per core on 8 NeuronCores.

Math: y[b,co,2m+p,2n+q] = bias[co]
        + sum_{i,j in {0,1}} sum_ci x[b,ci,m-i,n-j] * w[ci,co,p+2i,q+2j]
for parity class (p,q) in {0,1}^2, m,n in [0,33).

v2 (bf16 datapath): the 2e-2 rel-err gate leaves huge headroom, so x/w
are cast to bf16 on the host (rel err ~3e-3 incl. bf16 outputs).  This
halves input DMA, halves LDWEIGHTS, and -- critically -- avoids the
fp32 HIGH-power PE mode that made HAM duty-cycle the core to 50% for
the last 10us of the fp32r baseline.

Per image and parity class: output subgrid [128co x 33 x 33] computed
as 3 row-chunks of 11 rows; each chunk is one PSUM accumulation group
of 8 matmuls (2 ci-chunks x 4 taps (i,j)), K=128, M=128co, N=11*33=363
via a 2D strided rhs AP over a zero-padded 34x34 SBUF copy of x
(padded host-side), so no junk columns are computed.  PSUM->SBUF drain
fuses the bias add and writes a contiguous bf16 class tile; the parity
de-interleave is done on the HOST after gather (pure data movement).
Output leaves per class (278KB each, overlapped); the final class
leaves as 3 per-chunk DMAs on 3 queues so only ~90KB trails the last
matmul.  Drains rotate over vector/scalar/gpsimd so the tail drain
chain is parallel, and input DMA issues are scheduled per-engine in
consumption order (the first real matmul is gated only on one 128KB
weight class chunk + one 110KB x band).
"""

import numpy as np
import ml_dtypes

import concourse.bass as bass
import concourse.bacc as bacc
import concourse.tile as tile
from concourse import mybir
from concourse.bass_utils import run_bass_kernel_spmd

N_CORES = 8
B_PER = 2  # images per core

F32 = mybir.dt.float32
BF16 = mybir.dt.bfloat16

PW = 34            # padded x width (32 + 1 left + 1 right)
XLEN = PW * PW     # 1156 padded x elems per partition
XPAD = 1160        # sbuf/dram x free size (tail slack, keeps 4B align)
R = 11             # output parity rows per PSUM chunk
NCH = 3            # chunks: 3 * 11 = 33 parity rows
NW = 33            # useful output cols per parity row
NF = R * NW        # 363 matmul free dim (2D strided rhs AP)
NWARM = 2          # PE clock-ramp warmup matmuls (bridge until weights land)

CLASSES = [(0, 0), (0, 1), (1, 0), (1, 1)]


def _emit_group(nc, ps, wt, xv, p, q, r):
    """One PSUM accumulation group: 8 matmuls for class (p,q), chunk r.
    xv[c] is the [128, 34, 34] padded-image view of x for ci-chunk c."""
    k = 0
    for c in range(2):
        for i in range(2):
            for j in range(2):
                r0 = R * r - i + 1
                c0 = 1 - j
                nc.tensor.matmul(
                    ps[:],
                    wt[c][:, p, q, i, j, :],
                    xv[c][:, r0:r0 + R, c0:c0 + NW],
                    start=(k == 0),
                    stop=(k == 7),
                )
                k += 1


def build_nc(debug: bool = False) -> bass.Bass:
    nc = bacc.Bacc("TRN2", target_bir_lowering=False, debug=debug,
                   num_devices=N_CORES)

    # x arrives host-padded bf16: 34x34 zero-border layout + tail pad
    x_d = nc.declare_dram_parameter("x", [B_PER, 256, XPAD], BF16,
                                    isOutput=False)
    # w layout: [ci_chunk, p, q, ci, i, j, co] -- each (c, class) slice
    # is a CONTIGUOUS 128KB DRAM run so the DMA coalesces into big
    # packets (strided 1KB/partition rows cost ~35ns/row = ~4.5us/DMA)
    w_d = nc.declare_dram_parameter("w", [2, 2, 2, 128, 2, 2, 128], BF16,
                                    isOutput=False)
    b_d = nc.declare_dram_parameter("b", [128, 1], F32, isOutput=False)
    # class-major output: host de-interleaves parity grids
    y_d = nc.declare_dram_parameter("y", [B_PER, 2, 2, 128, NCH, R, NW],
                                    BF16, isOutput=True)

    with tile.TileContext(nc) as tc:
        with (
            tc.tile_pool(name="wp", bufs=2) as wpool,
            tc.tile_pool(name="bp", bufs=1) as bpool,
            tc.tile_pool(name="xp", bufs=2 * B_PER) as xpool,
            tc.tile_pool(name="cp", bufs=3) as cpool,
            tc.tile_pool(name="ps", bufs=7, space="PSUM") as ppool,
            tc.tile_pool(name="pw", bufs=1, space="PSUM") as warmpool,
        ):
            # --- PE clock-ramp warmup -------------------------------
            # HAM starts the PE slow and unthrottles after ~3us of
            # sustained activity; burn the input-DMA window on dummy
            # bf16 matmuls.  memset on gpsimd: its program loads first,
            # so the warmup starts ~1us earlier than via vector.
            wub = bpool.tile([128, 512], BF16)
            nc.gpsimd.memset(wub[:], 0.0)
            wps = warmpool.tile([128, 512], F32)
            for _ in range(NWARM):
                nc.tensor.matmul(wps[:], wub[:, 0:128], wub[:],
                                 start=True, stop=True)

            # --- tiles ----------------------------------------------
            wt = [wpool.tile([128, 2, 2, 2, 2, 128], BF16, name=f"wt{c}",
                             tag="wt")
                  for c in range(2)]
            bt = bpool.tile([128, 1], F32)
            xp = [[xpool.tile([128, XPAD], BF16, name=f"x{b}c{c}", tag="xt")
                   for c in range(2)] for b in range(B_PER)]
            xv = [[xp[b][c][:, 0:XLEN].rearrange("p (h w) -> p h w", w=PW)
                   for c in range(2)] for b in range(B_PER)]

            # --- input DMA issues, consumption order per engine -----
            # each dma_start costs ~650ns of sequencer time on the
            # issuing engine; all transfers are whole contiguous DRAM
            # runs.  The two first-matmul gates (w c0 class00, x0c0)
            # get scheduler priority 0 on separate engines.
            def dma_w(eng, c, p, q):
                eng.dma_start(out=wt[c][:, p, q], in_=w_d[c, p, q])

            def dma_x(eng, b, c):
                eng.dma_start(out=xp[b][c][:],
                              in_=x_d[b, c * 128:(c + 1) * 128])

            with tc.high_priority():
                dma_w(nc.sync, 0, 0, 0)
                dma_x(nc.scalar, 0, 0)
            # sync: bias + the other early weight classes
            nc.sync.dma_start(out=bt[:], in_=b_d[:])
            dma_w(nc.sync, 1, 0, 0)
            dma_w(nc.sync, 0, 0, 1)
            dma_w(nc.sync, 1, 0, 1)
            # scalar: x image0 c1, class (1,1) weights
            dma_x(nc.scalar, 0, 1)
            dma_w(nc.scalar, 0, 1, 1)
            dma_w(nc.scalar, 1, 1, 1)
            # gpsimd (after warmup memset): x image1, class (1,0) w
            dma_x(nc.gpsimd, 1, 0)
            dma_x(nc.gpsimd, 1, 1)
            dma_w(nc.gpsimd, 0, 1, 0)
            dma_w(nc.gpsimd, 1, 1, 0)

            # --- drains: alternate vector/scalar (gpsimd cannot read
            # PSUM) so consecutive chunk drains overlap ---------------
            def drain(r, ct, ps):
                in_ = ps[:].rearrange("p (m n) -> p m n", n=NW)
                out = ct[:, r]
                if r == 1:
                    nc.scalar.activation(out, in_,
                                         mybir.ActivationFunctionType.Identity,
                                         bias=bt[:], scale=1.0)
                else:
                    nc.vector.tensor_scalar_add(out, in_, bt[:])

            # --- main loops: class-major, chunk-major within class --
            out_engs = [nc.sync, nc.gpsimd, nc.scalar]
            oi = 0
            for b in range(B_PER):
                for (p, q) in CLASSES:
                    last = (b == B_PER - 1) and (p, q) == CLASSES[-1]
                    ct = cpool.tile([128, NCH, R, NW], BF16)
                    for r in range(NCH):
                        ps = ppool.tile([128, NF], F32)
                        _emit_group(nc, ps, wt, xv[b], p, q, r)
                        drain(r, ct, ps)
                        if last:
                            eng = out_engs[(oi + r) % 3]
                            eng.dma_start(out=y_d[b, p, q, :, r],
                                          in_=ct[:, r])
                    if not last:
                        eng = out_engs[oi % 3]
                        oi += 1
                        eng.dma_start(out=y_d[b, p, q], in_=ct[:])

    nc.compile()
    return nc


_nc_cache = None


def _get_nc():
    global _nc_cache
    if _nc_cache is None:
        _nc_cache = build_nc()
    return _nc_cache


def make_in_maps(x: np.ndarray, weight: np.ndarray, bias: np.ndarray):
    # w[ci,co,kh,kw] -> [c, p, q, ci', i, j, co], bf16
    w6 = (
        np.asarray(weight, dtype=np.float32)
        .reshape(2, 128, 128, 2, 2, 2, 2)      # [c, ci', co, i, p, j, q]
        .transpose(0, 4, 6, 1, 3, 5, 2)        # -> [c, p, q, ci', i, j, co]
    )
    w_host = np.ascontiguousarray(w6.astype(ml_dtypes.bfloat16))
    b_host = np.ascontiguousarray(
        np.asarray(bias, dtype=np.float32).reshape(128, 1)
    )
    x = np.asarray(x, dtype=np.float32)
    # host-side zero-pad into the 34x34(+tail) bf16 layout the kernel reads
    xpad = np.zeros((16, 256, XPAD), dtype=ml_dtypes.bfloat16)
    xpad[:, :, :XLEN].reshape(16, 256, PW, PW)[:, :, 1:33, 1:33] = \
        x.astype(ml_dtypes.bfloat16)
    return [
        {
            "x": np.ascontiguousarray(xpad[B_PER * i:B_PER * (i + 1)]),
            "w": w_host,
            "b": b_host,
        }
        for i in range(N_CORES)
    ]


def kernel(x: np.ndarray, weight: np.ndarray, bias: np.ndarray) -> np.ndarray:
    nc = _get_nc()
    in_maps = make_in_maps(x, weight, bias)
    res = run_bass_kernel_spmd(nc, in_maps, list(range(N_CORES)))
    out = np.empty((16, 128, 66, 66), dtype=np.float32)
    for i, r in enumerate(res.results):
        y = np.asarray(r["y"]).reshape(B_PER, 2, 2, 128, NCH * R, NW)
        for b in range(B_PER):
            for p in range(2):
                for q in range(2):
                    out[B_PER * i + b, :, p::2, q::2] = \
                        y[b, p, q].astype(np.float32)
    return out


# revision 13
# speedup vs baseline: 1.1059x; 1.0131x over previous
"""ConvTranspose2d (16,256,32,32) -> (16,128,66,66), stride 2, 4x4 kernel.

Strategy: data-parallel over batch, 2 images per core on 8 NeuronCores.

Math: y[b,co,2m+p,2n+q] = bias[co]
        + sum_{i,j in {0,1}} sum_ci x[b,ci,m-i,n-j] * w[ci,co,p+2i,q+2j]
for parity class (p,q) in {0,1}^2, m,n in [0,33).

v2 (bf16 datapath): the 2e-2 rel-err gate leaves huge headroom, so x/w
are cast to bf16 on the host (rel err ~3e-3 incl. bf16 outputs).  This
halves input DMA, halves LDWEIGHTS, and -- critically -- avoids the
fp32 HIGH-power PE mode that made HAM duty-cycle the core to 50% for
the last 10us of the fp32r baseline.

Per image and parity class: output subgrid [128co x 33 x 33] computed
as 3 row-chunks of 11 rows; each chunk is one PSUM accumulation group
of 8 matmuls (2 ci-chunks x 4 taps (i,j)), K=128, M=128co, N=11*33=363
via a 2D strided rhs AP over a zero-padded 34x34 SBUF copy of x
(padded host-side), so no junk columns are computed.  PSUM->SBUF drain
fuses the bias add and writes a contiguous bf16 class tile; the parity
de-interleave is done on the HOST after gather (pure data movement).
Output leaves per class (278KB each, overlapped); the final class
leaves as 3 per-chunk DMAs on 3 queues so only ~90KB trails the last
matmul.  Drains rotate over vector/scalar/gpsimd so the tail drain
chain is parallel, and input DMA issues are scheduled per-engine in
consumption order (the first real matmul is gated only on one 128KB
weight class chunk + one 110KB x band).
"""

import numpy as np
import ml_dtypes

import concourse.bass as bass
import concourse.bacc as bacc
import concourse.tile as tile
from concourse import mybir
from concourse.bass_utils import run_bass_kernel_spmd

N_CORES = 8
B_PER = 2  # images per core

F32 = mybir.dt.float32
BF16 = mybir.dt.bfloat16

PW = 34            # padded x width (32 + 1 left + 1 right)
XLEN = PW * PW     # 1156 padded x elems per partition
XPAD = 1160        # sbuf/dram x free size (tail slack, keeps 4B align)
R = 11             # output parity rows per PSUM chunk
NCH = 3            # chunks: 3 * 11 = 33 parity rows
NW = 33            # useful output cols per parity row
NF = R * NW        # 363 matmul free dim (2D strided rhs AP)
NWARM = 3          # PE clock-ramp warmup matmuls (bridge until weights land)

CLASSES = [(0, 0), (0, 1), (1, 0), (1, 1)]


def _emit_group(nc, ps, wt, xv, p, q, r):
    """One PSUM accumulation group: 8 matmuls for class (p,q), chunk r.
    xv[c] is the [128, 34, 34] padded-image view of x for ci-chunk c."""
    k = 0
    for c in range(2):
        for i in range(2):
            for j in range(2):
                r0 = R * r - i + 1
                c0 = 1 - j
                nc.tensor.matmul(
                    ps[:],
                    wt[c][:, p, q, i, j, :],
                    xv[c][:, r0:r0 + R, c0:c0 + NW],
                    start=(k == 0),
                    stop=(k == 7),
                )
                k += 1


def build_nc(debug: bool = False) -> bass.Bass:
    nc = bacc.Bacc("TRN2", target_bir_lowering=False, debug=debug,
                   num_devices=N_CORES)

    # x arrives host-padded bf16: 34x34 zero-border layout + tail pad
    x_d = nc.declare_dram_parameter("x", [B_PER, 256, XPAD], BF16,
                                    isOutput=False)
    # w layout: [ci_chunk, ci, p, q, i, j, co].  DMA cost is dominated
    # by per-partition-row packet count (~110ns/row under contention),
    # so each ci_chunk moves as ONE [128 x 2048B] DMA: 128 packets for
    # half the weights instead of 128 per class chunk.
    w_d = nc.declare_dram_parameter("w", [2, 128, 2, 2, 2, 2, 128], BF16,
                                    isOutput=False)
    b_d = nc.declare_dram_parameter("b", [128, 1], F32, isOutput=False)
    # class-major output: host de-interleaves parity grids
    y_d = nc.declare_dram_parameter("y", [B_PER, 2, 2, 128, NCH, R, NW],
                                    BF16, isOutput=True)

    with tile.TileContext(nc) as tc:
        with (
            tc.tile_pool(name="wp", bufs=2) as wpool,
            tc.tile_pool(name="bp", bufs=1) as bpool,
            tc.tile_pool(name="xp", bufs=2 * B_PER) as xpool,
            tc.tile_pool(name="cp", bufs=3) as cpool,
            tc.tile_pool(name="ps", bufs=7, space="PSUM") as ppool,
            tc.tile_pool(name="pw", bufs=1, space="PSUM") as warmpool,
        ):
            # --- PE clock-ramp warmup -------------------------------
            # HAM starts the PE slow and unthrottles after ~3us of
            # sustained activity; burn the input-DMA window on dummy
            # bf16 matmuls.  memset on gpsimd: its program loads first,
            # so the warmup starts ~1us earlier than via vector.
            wub = bpool.tile([128, 512], BF16)
            nc.gpsimd.memset(wub[:], 0.0)
            wps = warmpool.tile([128, 512], F32)
            for _ in range(NWARM):
                nc.tensor.matmul(wps[:], wub[:, 0:128], wub[:],
                                 start=True, stop=True)

            # --- tiles ----------------------------------------------
            wt = [wpool.tile([128, 2, 2, 2, 2, 128], BF16, name=f"wt{c}",
                             tag="wt")
                  for c in range(2)]
            bt = bpool.tile([128, 1], F32)
            xp = [[xpool.tile([128, XPAD], BF16, name=f"x{b}c{c}", tag="xt")
                   for c in range(2)] for b in range(B_PER)]
            xv = [[xp[b][c][:, 0:XLEN].rearrange("p (h w) -> p h w", w=PW)
                   for c in range(2)] for b in range(B_PER)]

            # --- input DMA issues: the 16 DMA engines round-robin all
            # active rings, so what gates the first matmul is the total
            # packet count in flight.  Wave 1 (priority 0): only the
            # two first-matmul gates (256 packets ~ 1.9us).  Wave 2:
            # the rest of image-0's needs.  Image-1's x is deferred to
            # program-mid so its 256 packets stay out of the window.
            def dma_x(eng, b, c):
                eng.dma_start(out=xp[b][c][:],
                              in_=x_d[b, c * 128:(c + 1) * 128])

            with tc.high_priority():
                nc.sync.dma_start(out=wt[0][:], in_=w_d[0])
                dma_x(nc.scalar, 0, 0)
            dma_x(nc.scalar, 0, 1)
            nc.sync.dma_start(out=wt[1][:], in_=w_d[1])
            nc.gpsimd.dma_start(out=bt[:], in_=b_d[:])

            # --- drains: alternate vector/scalar (gpsimd cannot read
            # PSUM) so consecutive chunk drains overlap ---------------
            def drain(r, ct, ps):
                in_ = ps[:].rearrange("p (m n) -> p m n", n=NW)
                out = ct[:, r]
                if r == 1:
                    nc.scalar.activation(out, in_,
                                         mybir.ActivationFunctionType.Identity,
                                         bias=bt[:], scale=1.0)
                else:
                    nc.vector.tensor_scalar_add(out, in_, bt[:])

            # --- main loops: class-major, chunk-major within class --
            out_engs = [nc.sync, nc.gpsimd, nc.scalar]
            oi = 0
            for b in range(B_PER):
                for ci_, (p, q) in enumerate(CLASSES):
                    last = (b == B_PER - 1) and (p, q) == CLASSES[-1]
                    ct = cpool.tile([128, NCH, R, NW], BF16)
                    for r in range(NCH):
                        ps = ppool.tile([128, NF], F32)
                        _emit_group(nc, ps, wt, xv[b], p, q, r)
                        drain(r, ct, ps)
                        if last:
                            eng = out_engs[(oi + r) % 3]
                            eng.dma_start(out=y_d[b, p, q, :, r],
                                          in_=ct[:, r])
                    if not last:
                        eng = out_engs[oi % 3]
                        oi += 1
                        eng.dma_start(out=y_d[b, p, q], in_=ct[:])
                    if b == 0 and ci_ == 0:
                        # image-1 x loads, out of the startup window
                        dma_x(nc.gpsimd, 1, 0)
                        dma_x(nc.gpsimd, 1, 1)

    nc.compile()
    return nc


_nc_cache = None


def _get_nc():
    global _nc_cache
    if _nc_cache is None:
        _nc_cache = build_nc()
    return _nc_cache


def make_in_maps(x: np.ndarray, weight: np.ndarray, bias: np.ndarray):
    # w[ci,co,kh,kw] -> [c, ci', p, q, i, j, co], bf16
    w6 = (
        np.asarray(weight, dtype=np.float32)
        .reshape(2, 128, 128, 2, 2, 2, 2)      # [c, ci', co, i, p, j, q]
        .transpose(0, 1, 4, 6, 3, 5, 2)        # -> [c, ci', p, q, i, j, co]
    )
    w_host = np.ascontiguousarray(w6.astype(ml_dtypes.bfloat16))
    b_host = np.ascontiguousarray(
        np.asarray(bias, dtype=np.float32).reshape(128, 1)
    )
    x = np.asarray(x, dtype=np.float32)
    # host-side zero-pad into the 34x34(+tail) bf16 layout the kernel reads
    xpad = np.zeros((16, 256, XPAD), dtype=ml_dtypes.bfloat16)
    xpad[:, :, :XLEN].reshape(16, 256, PW, PW)[:, :, 1:33, 1:33] = \
        x.astype(ml_dtypes.bfloat16)
    return [
        {
            "x": np.ascontiguousarray(xpad[B_PER * i:B_PER * (i + 1)]),
            "w": w_host,
            "b": b_host,
        }
        for i in range(N_CORES)
    ]


def kernel(x: np.ndarray, weight: np.ndarray, bias: np.ndarray) -> np.ndarray:
    nc = _get_nc()
    in_maps = make_in_maps(x, weight, bias)
    res = run_bass_kernel_spmd(nc, in_maps, list(range(N_CORES)))
    out = np.empty((16, 128, 66, 66), dtype=np.float32)
    for i, r in enumerate(res.results):
        y = np.asarray(r["y"]).reshape(B_PER, 2, 2, 128, NCH * R, NW)
        for b in range(B_PER):
            for p in range(2):
                for q in range(2):
                    out[B_PER * i + b, :, p::2, q::2] = \
                        y[b, p, q].astype(np.float32)
    return out


# revision 17
# speedup vs baseline: 1.1863x; 1.0727x over previous
"""ConvTranspose2d (16,256,32,32) -> (16,128,66,66), stride 2, 4x4 kernel.

Strategy: data-parallel over batch, 2 images per core on 8 NeuronCores.

Math: y[b,co,2m+p,2n+q] = bias[co]
        + sum_{i,j in {0,1}} sum_ci x[b,ci,m-i,n-j] * w[ci,co,p+2i,q+2j]
for parity class (p,q) in {0,1}^2, m,n in [0,33).

All-bf16 datapath (x, w, y; fp32 PSUM/bias): the 2e-2 rel-err gate
leaves huge headroom (measured ~3e-3), halves DMA, and avoids the
fp32-HIGH PE power mode.

Per image and parity class: output subgrid [128co x 33 x 33] as 3
row-chunks of 11 rows; each chunk is one PSUM group of 8 matmuls
(2 ci-chunks x 4 taps), K=128, M=128co, N=363 via a 2D strided rhs
over the zero-padded 34x34 SBUF x copy (padded host-side).  Drains
fuse the bias add, write contiguous bf16 class tiles; host does the
parity de-interleave.  Output leaves per class; the last class leaves
as 3 per-chunk DMAs so only ~90KB trails the final matmul.

Timing model (from traces): every [128-part, *] DMA is >=128 packets
at ~30ns+bytes/26GB/s per packet, served round-robin over 16 engines,
so the first matmul is gated by TOTAL packets in flight.  Wave 1 is
only w[c0,class00] (1KB rows) + x image0 c0.  Image-0 runs tap-major
(c-outer), pushing the x0c1/w-rest deadlines to matmul 13/25.  The
image-1 x loads are held back by a WAW gate (a 1-column write into
their tiles dependent on the first class-00 drain) so their 256
packets stay out of the startup window.  Bias rides a 1-packet
[1,128] DMA and is broadcast to [128,1] with a K=1 matmul.  The PE
clock ramps for ~6us from first activity (warmups bridge the DMA
window), and HAM grants full speed for a fixed ~30.7us window -- the
whole kernel is squeezed to fit most work inside it.
"""

import numpy as np
import ml_dtypes

import concourse.bass as bass
import concourse.bacc as bacc
import concourse.tile as tile
from concourse import mybir
from concourse.bass_utils import run_bass_kernel_spmd

N_CORES = 8
B_PER = 2  # images per core

F32 = mybir.dt.float32
BF16 = mybir.dt.bfloat16

PW = 34            # padded x width (32 + 1 left + 1 right)
XLEN = PW * PW     # 1156 padded x elems per partition
XPAD = 1160        # sbuf/dram x free size (tail slack, keeps 4B align)
R = 11             # output parity rows per PSUM chunk
NCH = 3            # chunks: 3 * 11 = 33 parity rows
NW = 33            # useful output cols per parity row
NF = R * NW        # 363 matmul free dim (2D strided rhs AP)
NWARM = 3          # PE clock-ramp warmup matmuls (bridge until inputs land)

CLASSES = [(0, 0), (0, 1), (1, 0), (1, 1)]
COPY = mybir.ActivationFunctionType.Copy
IDENT = mybir.ActivationFunctionType.Identity


def _mm(nc, ps, wt, xv, p, q, c, i, j, r, start, stop):
    r0 = R * r - i + 1
    c0 = 1 - j
    nc.tensor.matmul(
        ps[:],
        wt[c][:, 2 * p + q, (2 * i + j) * 128:(2 * i + j + 1) * 128],
        xv[c][:, r0:r0 + R, c0:c0 + NW],
        start=start,
        stop=stop,
        skip_group_check=True,
    )


def _emit_class_tapmajor(nc, pss, wt, xv, p, q):
    """Taps outer (c0 first: x0c1 not needed until matmul 13), chunks
    inner; the 3 PSUM groups accumulate interleaved."""
    k = 0
    for c in range(2):
        for i in range(2):
            for j in range(2):
                for r in range(NCH):
                    _mm(nc, pss[r], wt, xv, p, q, c, i, j, r,
                        start=(k == 0), stop=(k == 7))
                k += 1


def _emit_group(nc, ps, wt, xv, p, q, r):
    """Chunk-major: one PSUM group of 8 matmuls (early drains)."""
    k = 0
    for c in range(2):
        for i in range(2):
            for j in range(2):
                _mm(nc, ps, wt, xv, p, q, c, i, j, r,
                    start=(k == 0), stop=(k == 7))
                k += 1


def build_nc(debug: bool = False) -> bass.Bass:
    nc = bacc.Bacc("TRN2", target_bir_lowering=False, debug=debug,
                   num_devices=N_CORES)

    # x arrives host-padded bf16: 34x34 zero-border layout + tail pad
    x_d = nc.declare_dram_parameter("x", [B_PER, 256, XPAD], BF16,
                                    isOutput=False)
    # w layout: [ci_chunk, ci, class(2p+q), tap(2i+j)*co] -- contiguous
    # per-(c,class) DRAM runs
    w_d = nc.declare_dram_parameter("w", [2, 128, 4, 512], BF16,
                                    isOutput=False)
    b_d = nc.declare_dram_parameter("b", [1, 128], F32, isOutput=False)
    # class-major output: host de-interleaves parity grids
    y_d = nc.declare_dram_parameter("y", [B_PER, 2, 2, 128, NCH, R, NW],
                                    BF16, isOutput=True)

    with tile.TileContext(nc) as tc:
        with (
            tc.tile_pool(name="wp", bufs=2) as wpool,
            tc.tile_pool(name="bp", bufs=1) as bpool,
            tc.tile_pool(name="xp", bufs=2 * B_PER) as xpool,
            tc.tile_pool(name="cp", bufs=3) as cpool,
            tc.tile_pool(name="ps", bufs=6, space="PSUM") as ppool,
            tc.tile_pool(name="pw", bufs=1, space="PSUM") as warmpool,
        ):
            # --- tiles ----------------------------------------------
            wub = bpool.tile([128, 512], BF16)
            brt = bpool.tile([128, 128], F32)   # row 0: bias via 1 packet
            one = bpool.tile([128, 1], F32)
            bt = bpool.tile([128, 1], F32)
            wt = [wpool.tile([128, 4, 512], BF16, name=f"wt{c}", tag="wt")
                  for c in range(2)]
            xp = [[xpool.tile([128, XPAD], BF16, name=f"x{b}c{c}", tag="xt")
                   for c in range(2)] for b in range(B_PER)]
            xv = [[xp[b][c][:, 0:XLEN].rearrange("p (h w) -> p h w", w=PW)
                   for c in range(2)] for b in range(B_PER)]

            # --- warmup + bias broadcast ----------------------------
            # gpsimd's program loads first: memsets + the 1-packet bias
            # DMA go there so the PE can start at ~7.4us.
            nc.gpsimd.memset(wub[:], 0.0)
            nc.gpsimd.memset(one[:], 1.0)
            with tc.high_priority():
                nc.gpsimd.dma_start(out=brt[0:1, :], in_=b_d[:])
            wps = warmpool.tile([128, 512], F32)
            for _ in range(NWARM):
                nc.tensor.matmul(wps[:], wub[:, 0:128], wub[:],
                                 start=True, stop=True)
            # bias broadcast: [1,128] @ [1,1] -> PSUM [128,1]
            psb = warmpool.tile([128, 1], F32)
            nc.tensor.matmul(psb[:], brt[0:1, :], one[0:1, :],
                             start=True, stop=True)
            nc.scalar.activation(bt[:], psb[:], COPY)

            # --- input DMA waves ------------------------------------
            # wave 1 (priority 0): just the two first-matmul gates
            # (256 packets ~= 1.5us).
            with tc.high_priority():
                nc.sync.dma_start(out=wt[0][:, 0], in_=w_d[0, :, 0])
                nc.scalar.dma_start(out=xp[0][0][:], in_=x_d[0, 0:128])
            # wave 2: rest of image-0's inputs
            nc.scalar.dma_start(out=xp[0][1][:], in_=x_d[0, 128:256])
            nc.sync.dma_start(out=wt[0][:, 1:4], in_=w_d[0, :, 1:4])
            nc.sync.dma_start(out=wt[1][:], in_=w_d[1])

            # --- drains: alternate vector/scalar (gpsimd cannot read
            # PSUM) ---------------------------------------------------
            def drain(r, ct, ps):
                in_ = ps[:].rearrange("p (m n) -> p m n", n=NW)
                out = ct[:, r]
                if r == 1:
                    nc.scalar.activation(out, in_, IDENT, bias=bt[:],
                                         scale=1.0)
                else:
                    nc.vector.tensor_scalar_add(out, in_, bt[:])

            # --- main loops: class-major; image 0 tap-major ----------
            out_engs = [nc.sync, nc.gpsimd, nc.scalar]
            oi = 0
            for b in range(B_PER):
                for ci_, (p, q) in enumerate(CLASSES):
                    last = (b == B_PER - 1) and (p, q) == CLASSES[-1]
                    ct = cpool.tile([128, NCH, R, NW], BF16)
                    if b == 0:
                        pss = [ppool.tile([128, NF], F32, name="ps")
                               for _ in range(NCH)]
                        _emit_class_tapmajor(nc, pss, wt, xv[b], p, q)
                        for r in range(NCH):
                            drain(r, ct, pss[r])
                    else:
                        for r in range(NCH):
                            ps = ppool.tile([128, NF], F32)
                            _emit_group(nc, ps, wt, xv[b], p, q, r)
                            drain(r, ct, ps)
                            if last:
                                eng = out_engs[(oi + r) % 3]
                                eng.dma_start(out=y_d[b, p, q, :, r],
                                              in_=ct[:, r])
                    if not last:
                        eng = out_engs[oi % 3]
                        oi += 1
                        eng.dma_start(out=y_d[b, p, q], in_=ct[:])
                    if b == 0 and ci_ == 0:
                        # WAW gate: a 1-column write into the image-1 x
                        # tiles, dependent on the first class-00 drain,
                        # holds the 256-packet x1 DMAs out of the
                        # startup window (the scheduler hoists any
                        # dependency-free DMA onto an idle engine).
                        nc.scalar.activation(xp[1][0][:, 0:1],
                                             ct[:, 0, 0, 0:1], COPY)
                        nc.scalar.activation(xp[1][1][:, 0:1],
                                             ct[:, 0, 0, 0:1], COPY)
                        nc.gpsimd.dma_start(out=xp[1][0][:],
                                            in_=x_d[1, 0:128])
                        nc.gpsimd.dma_start(out=xp[1][1][:],
                                            in_=x_d[1, 128:256])

    nc.compile()
    return nc


_nc_cache = None


def _get_nc():
    global _nc_cache
    if _nc_cache is None:
        _nc_cache = build_nc()
    return _nc_cache


def make_in_maps(x: np.ndarray, weight: np.ndarray, bias: np.ndarray):
    # w[ci,co,kh,kw] -> [c, ci', class(2p+q), (2i+j)*co], bf16
    w6 = (
        np.asarray(weight, dtype=np.float32)
        .reshape(2, 128, 128, 2, 2, 2, 2)      # [c, ci', co, i, p, j, q]
        .transpose(0, 1, 4, 6, 3, 5, 2)        # -> [c, ci', p, q, i, j, co]
        .reshape(2, 128, 4, 512)
    )
    w_host = np.ascontiguousarray(w6.astype(ml_dtypes.bfloat16))
    b_host = np.ascontiguousarray(
        np.asarray(bias, dtype=np.float32).reshape(1, 128)
    )
    x = np.asarray(x, dtype=np.float32)
    # host-side zero-pad into the 34x34(+tail) bf16 layout the kernel reads
    xpad = np.zeros((16, 256, XPAD), dtype=ml_dtypes.bfloat16)
    xpad[:, :, :XLEN].reshape(16, 256, PW, PW)[:, :, 1:33, 1:33] = \
        x.astype(ml_dtypes.bfloat16)
    return [
        {
            "x": np.ascontiguousarray(xpad[B_PER * i:B_PER * (i + 1)]),
            "w": w_host,
            "b": b_host,
        }
        for i in range(N_CORES)
    ]


def kernel(x: np.ndarray, weight: np.ndarray, bias: np.ndarray) -> np.ndarray:
    nc = _get_nc()
    in_maps = make_in_maps(x, weight, bias)
    res = run_bass_kernel_spmd(nc, in_maps, list(range(N_CORES)))
    out = np.empty((16, 128, 66, 66), dtype=np.float32)
    for i, r in enumerate(res.results):
        y = np.asarray(r["y"]).reshape(B_PER, 2, 2, 128, NCH * R, NW)
        for b in range(B_PER):
            for p in range(2):
                for q in range(2):
                    out[B_PER * i + b, :, p::2, q::2] = \
                        y[b, p, q].astype(np.float32)
    return out
